# revision 4
# baseline (speedup 1.0000x reference)
"""Trainium2 Bass kernel for nn_EnhanceDiversityFeatureExtracition.

Computes  loss = mean((x-y)^2) + ALPHA * diversity_reg(conv_w)
where diversity_reg builds a 64x64 Gram matrix of the F=64 slices
conv_w[:, :, i, :] (each flattened to a 786432-vector), normalizes it to
cosine similarities, and sums the entries with tau < sim <= 1 off the
diagonal.

Distribution (8 NeuronCores, SPMD):
  - x_batch / y_batch sharded on batch dim: 256 rows per core.
  - conv_w viewed as A = conv_w.reshape(262144, 192)  (row m = (o,c),
    col = f*3+k).  gram[i,j] = sum_m sum_k A[m,3i+k]*A[m,3j+k], so A is
    sharded along the 262144-row reduction axis: 32768 rows per core.
  - Each core returns a partial 64x64 gram and per-partition partial
    sums of (x-y)^2 in one merged output tensor; the host sums the
    partials and applies the tiny 64x64 masked-similarity epilogue.

On-core dataflow (DMA-roofline bound: 32 MiB/core of HBM reads):
  - A shard streams in 32 blocks of 1024 rows laid out as
    [128 partitions x 1536 floats] (per-partition contiguous 6KB lines),
    alternating between the two HWDGE rings (sync / scalar).
  - Each block yields 8 sub-tiles; sub-tiles t=0..6 use a 256-wide
    fp32r moving operand at full rate accumulating C = A^T A into
    cps1/cps2.  t=7 would run out of bounds, so its moving window is
    shifted left by 64 and accumulated into separate psum tiles
    (cpsS1/cpsS2) whose columns 64:256 hold the real contribution; the
    shift is folded into the epilogue adds for free.  No padding or
    memset needed.
  - x/y stream as 8+8 chunks of [128 x 1024] via SWDGE (gpsimd), one
    chunk per block over the first 16 blocks, so the MSE traffic is
    spread evenly and the tensor engine never idles long enough for the
    HAM clock gate to throttle it.  DVE computes d = x-y, ACT Square
    accumulates per-partition partials straight into the output tile.
  - Epilogue: csb = cps + shifted cpsS (DVE adds), six selection
    matmuls extract gram[i,j] = sum_k C[3i+k,3j+k], and a single merged
    DMA returns [128, 72]: cols 0:64 = gram rows (partitions 0:64),
    cols 64:72 = MSE partial sums.
"""

import numpy as np

import concourse.bass as bass
import concourse.mybir as mybir
from concourse import bacc, tile
from concourse.bass_utils import run_bass_kernel_spmd

N_CORES = 8
B, D = 2048, 4096            # x_batch / y_batch
M, G = 262144, 192           # conv_w as (M, G); G = F*KW
F, KW = 64, 3
ROWS = B // N_CORES          # 256 batch rows per core
MC = M // N_CORES            # 32768 reduction rows per core
TPB = 8                      # 128-row tiles per DMA block
BLK = 128 * TPB              # 1024 rows per block
NBLK = MC // BLK             # 32
NCH = 8                      # MSE chunks per operand per core
CHW = (ROWS * D) // (128 * NCH)  # 1024 floats per partition per chunk

ALPHA = 0.0005
TAU = 0.2

_prog = None


def _build() -> bass.Bass:
    nc = bacc.Bacc(None, target_bir_lowering=False)
    f32 = mybir.dt.float32
    f32r = mybir.dt.float32r

    xs = nc.dram_tensor("xs", [ROWS, D], f32, kind="ExternalInput")
    ys = nc.dram_tensor("ys", [ROWS, D], f32, kind="ExternalInput")
    aw = nc.dram_tensor("aw", [MC, G], f32r, kind="ExternalInput")
    out = nc.dram_tensor("out", [128, F + NCH], f32, kind="ExternalOutput")

    # Selection matrix for the k-diagonal extraction:
    # S[3j+k, 64k+j] = 1, so (S^T C S)-style products give
    # gram[i,j] = sum_k C[3i+k, 3j+k].
    S = np.zeros((G, G), np.float32)
    for k in range(KW):
        for j in range(F):
            S[KW * j + k, F * k + j] = 1.0
    s_dram = nc.inline_tensor(S, name="sel_const")

    RW = 256  # moving operand width for the fp32r full-rate mode

    with tile.TileContext(nc) as tc:
        with (
            tc.tile_pool(name="apool", bufs=12) as apool,
            tc.tile_pool(name="xpool", bufs=2) as xpool,
            tc.tile_pool(name="ypool", bufs=2) as ypool,
            tc.tile_pool(name="dpool", bufs=2) as dpool,
            tc.tile_pool(name="opool", bufs=1) as opool,
            tc.tile_pool(name="spool", bufs=1) as spool,
            tc.tile_pool(name="psum", bufs=1, space=bass.MemorySpace.PSUM) as psum,
        ):
            # C = A^T A accumulators; rows 0-127 (cps1) and 128-191 (cps2).
            # cpsS* take the shifted t=7 contribution (cols 64:256 real).
            cps1 = psum.tile([128, RW], f32, tag="cps1")
            cps2 = psum.tile([F, RW], f32, tag="cps2")
            cpsS1 = psum.tile([128, RW], f32, tag="cpsS1")
            cpsS2 = psum.tile([F, RW], f32, tag="cpsS2")

            # merged output: cols 0:64 gram (rows 0:64), cols 64:72 sse
            otile = opool.tile([128, F + NCH], f32, tag="otile")
            nc.vector.memset(otile[F:128, 0:F], 0.0)

            # selection matrix up front so it never waits behind the
            # streaming DMAs in a HWDGE FIFO
            ssb1r = spool.tile([128, G], f32, tag="ssb1r")
            nc.scalar.dma_start(ssb1r[:], s_dram[0:128, :])
            ssb1 = spool.tile([128, G], f32r, tag="ssb1")
            nc.vector.tensor_copy(ssb1[:], ssb1r[:])
            ssb2r = spool.tile([F, G], f32, tag="ssb2r")
            nc.scalar.dma_start(ssb2r[:], s_dram[128:G, :])
            ssb2 = spool.tile([F, G], f32r, tag="ssb2")
            nc.vector.tensor_copy(ssb2[:], ssb2r[:])

            # per-partition contiguous views
            awv = aw[:].rearrange("(b p t) g -> b p (t g)", p=128, t=TPB)
            xv = xs[:].rearrange("(p t) d -> p (t d)", p=128)
            yv = ys[:].rearrange("(p t) d -> p (t d)", p=128)

            n_t = NBLK * (TPB - 1)
            n_s = NBLK
            ti = 0
            si = 0
            for b in range(NBLK):
                at = apool.tile([128, TPB * G], f32r, tag="at")
                eng = nc.sync if (b % 2 == 0) else nc.scalar
                eng.dma_start(at[:], awv[b])
                for t in range(TPB - 1):
                    rhs = at[:, t * G:t * G + RW]
                    w1 = at[:, t * G:t * G + 128]
                    w2 = at[:, t * G + 128:t * G + G]
                    nc.tensor.matmul(
                        cps1[:], w1, rhs,
                        start=(ti == 0), stop=(ti == n_t - 1),
                    )
                    nc.tensor.matmul(
                        cps2[:], w2, rhs,
                        start=(ti == 0), stop=(ti == n_t - 1),
                    )
                    ti += 1
                # t = TPB-1: shift the moving window left by 64 so it stays
                # in bounds; real output lands in psum cols 64:256.
                t = TPB - 1
                rhs = at[:, t * G - 64:t * G + G]
                w1 = at[:, t * G:t * G + 128]
                w2 = at[:, t * G + 128:t * G + G]
                nc.tensor.matmul(
                    cpsS1[:], w1, rhs,
                    start=(si == 0), stop=(si == n_s - 1),
                )
                nc.tensor.matmul(
                    cpsS2[:], w2, rhs,
                    start=(si == 0), stop=(si == n_s - 1),
                )
                si += 1

                # one 512KB x or y chunk per block over the first 16 blocks
                if b < 2 * NCH:
                    ch = b // 2
                    if b % 2 == 0:
                        xt = xpool.tile([128, CHW], f32, tag="xt")
                        nc.gpsimd.dma_start(xt[:], xv[:, ch * CHW:(ch + 1) * CHW])
                    else:
                        yt = ypool.tile([128, CHW], f32, tag="yt")
                        nc.gpsimd.dma_start(yt[:], yv[:, ch * CHW:(ch + 1) * CHW])
                        dtile = dpool.tile([128, CHW], f32, tag="dt")
                        nc.vector.tensor_sub(dtile[:], xt[:], yt[:])
                        nc.scalar.activation(
                            dtile[:], dtile[:],
                            mybir.ActivationFunctionType.Square,
                            accum_out=otile[:, F + ch:F + ch + 1],
                        )

            # ---- epilogue: fold the shifted accumulators in during the
            # PSUM->SBUF move, then extract gram[i,j] = sum_k C[3i+k, 3j+k]
            csbS1 = opool.tile([128, G], f32, tag="csbS1")
            nc.vector.tensor_copy(csbS1[:], cpsS1[:, 64:RW])
            csb1 = opool.tile([128, G], f32r, tag="csb1")
            nc.vector.tensor_add(csb1[:], cps1[:, :G], csbS1[:])
            csbS2 = opool.tile([F, G], f32, tag="csbS2")
            nc.vector.tensor_copy(csbS2[:], cpsS2[:, 64:RW])
            csb2 = opool.tile([F, G], f32r, tag="csb2")
            nc.vector.tensor_add(csb2[:], cps2[:, :G], csbS2[:])

            gps = psum.tile([F, F], f32, tag="gps")
            n_sel = 2 * KW
            gi = 0
            for k in range(KW):
                for ssb, csb in ((ssb1, csb1), (ssb2, csb2)):
                    nc.tensor.matmul(
                        gps[:],
                        ssb[:, F * k:F * (k + 1)],
                        csb[:, k::KW],
                        start=(gi == 0), stop=(gi == n_sel - 1),
                    )
                    gi += 1

            nc.vector.tensor_copy(otile[0:F, 0:F], gps[:])
            nc.sync.dma_start(out[:], otile[:])

    nc.finalize()
    return nc


def _get_prog() -> bass.Bass:
    global _prog
    if _prog is None:
        _prog = _build()
    return _prog


def _epilogue(gram: np.ndarray, sse: float) -> np.ndarray:
    norms = np.sqrt(np.diag(gram))
    sim = gram / np.outer(norms, norms)
    mask = (sim > TAU) & (sim <= 1.0) & (~np.eye(F, dtype=bool))
    reg = sim[mask].sum()
    loss = sse / float(B * D) + ALPHA * reg
    return np.asarray(np.float32(loss))


def kernel(x_batch: np.ndarray, y_batch: np.ndarray, conv_w: np.ndarray) -> np.ndarray:
    nc = _get_prog()
    A = np.ascontiguousarray(conv_w.reshape(M, G))
    in_maps = []
    for c in range(N_CORES):
        in_maps.append({
            "xs": np.ascontiguousarray(x_batch[c * ROWS:(c + 1) * ROWS]),
            "ys": np.ascontiguousarray(y_batch[c * ROWS:(c + 1) * ROWS]),
            "aw": np.ascontiguousarray(A[c * MC:(c + 1) * MC]),
        })
    res = run_bass_kernel_spmd(nc, in_maps, core_ids=list(range(N_CORES))).results
    gram = np.zeros((F, F), np.float64)
    sse = 0.0
    for r in res:
        o = r["out"]
        gram += o[:F, :F].astype(np.float64)
        sse += float(o[:, F:F + NCH].sum(dtype=np.float64))
    return _epilogue(gram, sse)


# revision 6
# speedup vs baseline: 1.2086x; 1.2086x over previous
"""Trainium2 Bass kernel for nn_EnhanceDiversityFeatureExtracition.

Computes  loss = mean((x-y)^2) + ALPHA * diversity_reg(conv_w)
where diversity_reg builds a 64x64 Gram matrix of the F=64 slices
conv_w[:, :, i, :] (each flattened to a 786432-vector), normalizes it to
cosine similarities, and sums the entries with tau < sim <= 1 off the
diagonal.

Distribution (8 NeuronCores, SPMD):
  - x_batch / y_batch sharded on batch dim: 256 rows per core.
  - conv_w viewed as A = conv_w.reshape(262144, 192)  (row m = (o,c),
    col = f*3+k).  gram[i,j] = sum_m sum_k A[m,3i+k]*A[m,3j+k], so A is
    sharded along the 262144-row reduction axis: 32768 rows per core.
  - Each core returns a partial 64x64 gram and per-partition partial
    sums of (x-y)^2 in one merged output tensor; the host sums the
    partials and applies the tiny 64x64 masked-similarity epilogue.

On-core dataflow (DMA-roofline bound: 32 MiB/core of HBM reads):
  - All DMAs issue from the single sync HWDGE ring: one logical queue
    keeps each SDMA engine streaming sequential addresses (measured
    ~24 GB/s/engine; spreading across queues costs ~20%).
  - A shard streams in 15 blocks of 2048 rows ([128 x 3072] fp32,
    12 KiB per-partition lines = 3 aligned 4KiB packets) plus a
    1024-row and two 512-row tail blocks so the PE drains quickly
    after the last byte lands.
  - Each block tile carries 64 junk columns past the real data so the
    last sub-tile's 256-wide fp32r moving operand stays in bounds;
    the junk only feeds PSUM columns 192:255, which are never read.
    No memset needed.
  - Per 128-row sub-tile: two fp32r full-rate matmuls (stationary
    cols 0:128 / 128:192, moving 256) accumulate C = A^T A into
    cps1/cps2 across the whole shard.
  - x/y stream as 16 chunks of [128 x 1024] interleaved one per conv
    block over the first 16 blocks, so MSE traffic is spread evenly
    and the tensor engine never idles past the HAM 3.4us window.
    DVE computes d = x-y; ACT Square accumulates per-partition
    partials straight into the output tile.
  - A short burst of throwaway matmuls on the selection-matrix tile
    fires while the first conv block is still in flight, so the HAM
    clock gate releases (1.2 -> 2.4 GHz) before the real matmuls
    begin.
  - Epilogue: six selection matmuls extract
    gram[i,j] = sum_k C[3i+k, 3j+k], and a single merged DMA returns
    [128, 72]: cols 0:64 = gram rows (partitions 0:64), cols 64:72 =
    MSE partial sums.
"""

import numpy as np

import concourse.bass as bass
import concourse.mybir as mybir
from concourse import bacc, tile
from concourse.bass_utils import run_bass_kernel_spmd

N_CORES = 8
B, D = 2048, 4096            # x_batch / y_batch
M, G = 262144, 192           # conv_w as (M, G); G = F*KW
F, KW = 64, 3
ROWS = B // N_CORES          # 256 batch rows per core
MC = M // N_CORES            # 32768 reduction rows per core
# tiles-per-block schedule: 15 big blocks + progressively smaller tail
TPBS = [16] * 15 + [8, 4, 4]
assert sum(TPBS) * 128 == MC
NCH = 8                      # MSE chunks per operand per core
CHW = (ROWS * D) // (128 * NCH)  # 1024 floats per partition per chunk
PAD = 64                     # junk cols so the last moving operand fits

ALPHA = 0.0005
TAU = 0.2

_prog = None


def _build() -> bass.Bass:
    nc = bacc.Bacc(None, target_bir_lowering=False)
    f32 = mybir.dt.float32
    f32r = mybir.dt.float32r

    xs = nc.dram_tensor("xs", [ROWS, D], f32, kind="ExternalInput")
    ys = nc.dram_tensor("ys", [ROWS, D], f32, kind="ExternalInput")
    aw = nc.dram_tensor("aw", [MC, G], f32r, kind="ExternalInput")
    out = nc.dram_tensor("out", [128, F + NCH], f32, kind="ExternalOutput")

    # Selection matrix for the k-diagonal extraction:
    # S[3j+k, 64k+j] = 1, so (S^T C S)-style products give
    # gram[i,j] = sum_k C[3i+k, 3j+k].
    S = np.zeros((G, G), np.float32)
    for k in range(KW):
        for j in range(F):
            S[KW * j + k, F * k + j] = 1.0
    s_dram = nc.inline_tensor(S, name="sel_const")

    RW = 256  # moving operand width for the fp32r full-rate mode

    with tile.TileContext(nc) as tc:
        with (
            tc.tile_pool(name="apool", bufs=8) as apool,
            tc.tile_pool(name="xpool", bufs=2) as xpool,
            tc.tile_pool(name="ypool", bufs=2) as ypool,
            tc.tile_pool(name="dpool", bufs=2) as dpool,
            tc.tile_pool(name="opool", bufs=1) as opool,
            tc.tile_pool(name="spool", bufs=1) as spool,
            tc.tile_pool(name="psum", bufs=1, space=bass.MemorySpace.PSUM) as psum,
        ):
            # C = A^T A accumulators; rows 0-127 (cps1) and 128-191 (cps2)
            cps1 = psum.tile([128, RW], f32, tag="cps1")
            cps2 = psum.tile([F, RW], f32, tag="cps2")

            # merged output: cols 0:64 gram (rows 0:64), cols 64:72 sse
            otile = opool.tile([128, F + NCH], f32, tag="otile")

            # selection matrix up front (first in the sync DMA FIFO)
            ssb1r = spool.tile([128, G], f32, tag="ssb1r")
            nc.sync.dma_start(ssb1r[:], s_dram[0:128, :])
            ssb2r = spool.tile([F, G], f32, tag="ssb2r")
            nc.sync.dma_start(ssb2r[:], s_dram[128:G, :])
            ssb1 = spool.tile([128, G], f32r, tag="ssb1")
            nc.vector.tensor_copy(ssb1[:], ssb1r[:])
            ssb2 = spool.tile([F, G], f32r, tag="ssb2")
            nc.vector.tensor_copy(ssb2[:], ssb2r[:])

            # HAM warmup: throwaway matmuls on the selection tile keep the
            # PE busy while conv block 0 is in flight, so the clock gate
            # opens before the real work. wps is never read.
            wps = psum.tile([F, F], f32, tag="wps")
            for w in range(16):
                nc.tensor.matmul(
                    wps[:], ssb1[:, 0:F], ssb1[:, 0:F],
                    start=(w == 0), stop=(w == 15),
                )

            # per-partition contiguous views
            xv = xs[:].rearrange("(p t) d -> p (t d)", p=128)
            yv = ys[:].rearrange("(p t) d -> p (t d)", p=128)

            n_t = sum(TPBS)
            ti = 0
            r0 = 0
            for b, tpb in enumerate(TPBS):
                at = apool.tile([128, 16 * G + PAD], f32r, tag="at")
                awb = aw[r0:r0 + 128 * tpb, :].rearrange(
                    "(p t) g -> p (t g)", p=128
                )
                nc.sync.dma_start(at[:, :tpb * G], awb)
                r0 += 128 * tpb

                # one 512KB x or y chunk per block over the first 16 blocks
                if b < 2 * NCH:
                    ch = b // 2
                    if b % 2 == 0:
                        xt = xpool.tile([128, CHW], f32, tag="xt")
                        nc.sync.dma_start(xt[:], xv[:, ch * CHW:(ch + 1) * CHW])
                    else:
                        yt = ypool.tile([128, CHW], f32, tag="yt")
                        nc.sync.dma_start(yt[:], yv[:, ch * CHW:(ch + 1) * CHW])

                for t in range(tpb):
                    rhs = at[:, t * G:t * G + RW]
                    w1 = at[:, t * G:t * G + 128]
                    w2 = at[:, t * G + 128:t * G + G]
                    nc.tensor.matmul(
                        cps1[:], w1, rhs,
                        start=(ti == 0), stop=(ti == n_t - 1),
                    )
                    nc.tensor.matmul(
                        cps2[:], w2, rhs,
                        start=(ti == 0), stop=(ti == n_t - 1),
                    )
                    ti += 1

                if b < 2 * NCH and b % 2 == 1:
                    ch = b // 2
                    dtile = dpool.tile([128, CHW], f32, tag="dt")
                    nc.vector.tensor_sub(dtile[:], xt[:], yt[:])
                    nc.scalar.activation(
                        dtile[:], dtile[:],
                        mybir.ActivationFunctionType.Square,
                        accum_out=otile[:, F + ch:F + ch + 1],
                    )

            # ---- extract gram[i,j] = sum_k C[3i+k, 3j+k] via selection
            csb1 = opool.tile([128, G], f32r, tag="csb1")
            nc.vector.tensor_copy(csb1[:], cps1[:, :G])
            csb2 = opool.tile([F, G], f32r, tag="csb2")
            nc.vector.tensor_copy(csb2[:], cps2[:, :G])

            gps = psum.tile([F, F], f32, tag="gps")
            n_sel = 2 * KW
            gi = 0
            for k in range(KW):
                for ssb, csb in ((ssb1, csb1), (ssb2, csb2)):
                    nc.tensor.matmul(
                        gps[:],
                        ssb[:, F * k:F * (k + 1)],
                        csb[:, k::KW],
                        start=(gi == 0), stop=(gi == n_sel - 1),
                    )
                    gi += 1

            nc.vector.tensor_copy(otile[0:F, 0:F], gps[:])
            nc.sync.dma_start(out[:], otile[:])

    nc.finalize()
    return nc


def _get_prog() -> bass.Bass:
    global _prog
    if _prog is None:
        _prog = _build()
    return _prog


def _epilogue(gram: np.ndarray, sse: float) -> np.ndarray:
    norms = np.sqrt(np.diag(gram))
    sim = gram / np.outer(norms, norms)
    mask = (sim > TAU) & (sim <= 1.0) & (~np.eye(F, dtype=bool))
    reg = sim[mask].sum()
    loss = sse / float(B * D) + ALPHA * reg
    return np.asarray(np.float32(loss))


def kernel(x_batch: np.ndarray, y_batch: np.ndarray, conv_w: np.ndarray) -> np.ndarray:
    nc = _get_prog()
    A = np.ascontiguousarray(conv_w.reshape(M, G))
    in_maps = []
    for c in range(N_CORES):
        in_maps.append({
            "xs": np.ascontiguousarray(x_batch[c * ROWS:(c + 1) * ROWS]),
            "ys": np.ascontiguousarray(y_batch[c * ROWS:(c + 1) * ROWS]),
            "aw": np.ascontiguousarray(A[c * MC:(c + 1) * MC]),
        })
    res = run_bass_kernel_spmd(nc, in_maps, core_ids=list(range(N_CORES))).results
    gram = np.zeros((F, F), np.float64)
    sse = 0.0
    for r in res:
        o = r["out"]
        gram += o[:F, :F].astype(np.float64)
        sse += float(o[:, F:F + NCH].sum(dtype=np.float64))
    return _epilogue(gram, sse)
